# revision 22
# baseline (speedup 1.0000x reference)
"""GQA attention (B=1, S=2048, D=4096, HQ=32, HKV=8, HD=128) + RoPE + causal,
tensor-parallel over heads on 8 TRN2 NeuronCores.

Layout strategy (host pre-lays-out everything feature-major so the device
never transposes activations):
  - xT [D, S] bf16; per-core wq/wk columns permuted even|odd per head so RoPE
    pairs become partition halves (wq pre-scaled by 1/sqrt(HD)); cos/sin
    transposed+duplicated to [128, S] f32.
  - RoPE: rope(q) = cos*q + SWAP(sin*q) where SWAP is a signed half-swap
    permutation applied on the PE.
  - phase0: k/vT for the full sequence (one contiguous xT row-block DMA per
    k-tile, 8 psum accumulators), vT -> v via PE transpose.
  - per sq-tile q: q-projections (4 heads), then causally-skipped attention
    with scoresT = kT.T@qT per 128-sk block (ragged diagonal blocks, mask
    added from the host-transposed diag mask), exp on ACT software-pipelined
    one block deep so the PE never waits, PV accumulates outT[d, sq], row
    sums via an all-ones stationary matmul (already broadcast), reciprocal +
    one DVE mul to normalize.
  - After each sq-tile an AllGather ships outT[:, tile] (bf16) to every core;
    wo is column-sharded 8 ways (resident 4MB bf16), and wo(q) runs between
    the next tile's projections and attention so the AG latency stays hidden.
  - All PE matmuls in bf16 (f32 PSUM accumulation); rel err ~4e-3.
"""

import math

import ml_dtypes
import numpy as np

import concourse.bass as bass
import concourse.tile as tile
from concourse import bacc, mybir
from concourse.bass_utils import run_bass_kernel_spmd

F32 = mybir.dt.float32
BF16 = mybir.dt.bfloat16

S = 2048
D = 4096
HQ, HKV, HD = 32, 8, 128
HL = HQ // 8            # 4 local q heads per core
SQT = 512               # sq tile
NSQ = S // SQT          # 4
NK = D // 128           # 32 contraction k-tiles
NSK = S // 128          # 16 sk tiles
N_CORES = 8
AF = mybir.ActivationFunctionType
ALU = mybir.AluOpType


def build_nc():
    nc = bacc.Bacc(num_devices=N_CORES, num_swdge_queues=4)

    xT = nc.declare_dram_parameter("xT", [D, S], BF16, isOutput=False)
    wq = nc.declare_dram_parameter("wq", [D, HL * HD], BF16, isOutput=False)
    wkv = nc.declare_dram_parameter("wkv", [D, 2 * HD], BF16, isOutput=False)
    wo = nc.declare_dram_parameter("wo", [D, SQT], BF16, isOutput=False)  # col shard
    cos2 = nc.declare_dram_parameter("cos2", [128, S], F32, isOutput=False)
    sin2 = nc.declare_dram_parameter("sin2", [128, S], F32, isOutput=False)
    mtd = nc.declare_dram_parameter("mtd", [NSK, 128, 128], F32, isOutput=False)
    swp = nc.declare_dram_parameter("swp", [128, 128], BF16, isOutput=False)
    idn = nc.declare_dram_parameter("idn", [128, 128], BF16, isOutput=False)
    out = nc.declare_dram_parameter("out", [S, SQT], F32, isOutput=True)

    with tile.TileContext(nc) as tc:
        with tc.tile_pool(name="const", bufs=1) as constp, \
             tc.tile_pool(name="wqp", bufs=1) as wqp, \
             tc.tile_pool(name="wkvp", bufs=1) as wkvp, \
             tc.tile_pool(name="wob", bufs=1) as wob, \
             tc.tile_pool(name="xfull", bufs=3) as xfull, \
             tc.tile_pool(name="xp", bufs=8) as xp, \
             tc.tile_pool(name="qp", bufs=2) as qp, \
             tc.tile_pool(name="ep", bufs=4) as ep, \
             tc.tile_pool(name="tp", bufs=2) as tp, \
             tc.tile_pool(name="agp", bufs=2) as agp, \
             tc.tile_pool(name="op", bufs=2) as opool, \
             tc.tile_pool(name="outp", bufs=2) as outp, \
             tc.tile_pool(name="dram", bufs=1, space="DRAM") as dramp:

            # ---- phase0 weights: 4 k-tiles per DMA, wk|wv concatenated ----
            wkv_t = [wkvp.tile([128, 4, 2 * HD], BF16, tag=f"wkv{i}",
                               name=f"wkv{i}") for i in range(8)]
            for i in range(8):
                nc.scalar.dma_start(
                    out=wkv_t[i],
                    in_=wkv[i * 512:(i + 1) * 512, :].rearrange(
                        "(t p) c -> p t c", p=128))

            # ---- constants ----
            cos_sb = constp.tile([128, S], F32)
            nc.scalar.dma_start(out=cos_sb, in_=cos2[:, :])
            sin_sb = constp.tile([128, S], F32)
            nc.scalar.dma_start(out=sin_sb, in_=sin2[:, :])
            mtd_sb = constp.tile([128, NSK, 128], F32)
            nc.scalar.dma_start(out=mtd_sb, in_=mtd[:, :, :].transpose([1, 0, 2]))
            swp_sb = constp.tile([128, 128], BF16)
            nc.scalar.dma_start(out=swp_sb, in_=swp[:, :])
            idn_sb = constp.tile([128, 128], BF16)
            nc.scalar.dma_start(out=idn_sb, in_=idn[:, :])
            ones_f = constp.tile([128, 128], F32)
            nc.vector.memset(ones_f, 1.0)
            allones = constp.tile([128, 128], BF16)
            nc.scalar.activation(allones, ones_f, AF.Copy)

            kT_sb = constp.tile([128, S], BF16)      # kv head, feature-major
            v_sb = constp.tile([128, S], BF16)       # [sk%128, (sk//128)*128 + d]

            wq_t = [wqp.tile([128, HL * HD], BF16, tag=f"wq{kt}", name=f"wq{kt}")
                    for kt in range(NK)]

            wo_sb = wob.tile([128, NK, SQT], BF16)
            nc.gpsimd.dma_start(
                out=wo_sb, in_=wo[:, :].rearrange("(kt p) n -> p kt n", p=128))

            ag_in = [[dramp.tile([128, SQT], BF16, name=f"agin{q}_{h}")
                      for h in range(HL)] for q in range(NSQ)]
            ag_out = [[dramp.tile([8 * 128, SQT], BF16, addr_space="Shared",
                                  name=f"agout{q}_{h}") for h in range(HL)]
                      for q in range(NSQ)]

            def rope(ps, cos_cols, sin_cols, dst, psum_pool, ptag):
                """dst[bf16 sbuf 128xSQT] = cos*ps + signed-half-swap(sin*ps)."""
                t1 = tp.tile([128, SQT], F32, tag="t1")
                nc.vector.tensor_tensor(out=t1, in0=ps, in1=cos_cols, op=ALU.mult)
                t2 = tp.tile([128, SQT], BF16, tag="t2")
                nc.vector.tensor_tensor(out=t2, in0=ps, in1=sin_cols, op=ALU.mult)
                t2s = psum_pool.tile([128, SQT], F32, tag=ptag, name="t2s")
                nc.tensor.matmul(t2s, swp_sb, t2, start=True, stop=True)
                nc.vector.tensor_tensor(out=dst, in0=t1, in1=t2s, op=ALU.add)

            # ================= phase 0: k and v for the full sequence ==========
            with tc.tile_pool(name="pskv", bufs=8, space="PSUM") as pskv:
                k_ps = [pskv.tile([128, SQT], F32, tag="pkv", name=f"kps{c}")
                        for c in range(NSQ)]
                v_ps = [pskv.tile([128, SQT], F32, tag="pkv", name=f"vps{c}")
                        for c in range(NSQ)]
                for kt in range(NK):
                    xt = xfull.tile([128, S], BF16, tag="xf")
                    nc.sync.dma_start(out=xt, in_=xT[kt * 128:(kt + 1) * 128, :])
                    nc.sync.dma_start(out=wq_t[kt],
                                      in_=wq[kt * 128:(kt + 1) * 128, :])
                    for c in range(NSQ):
                        nc.tensor.matmul(k_ps[c], wkv_t[kt // 4][:, kt % 4, 0:HD],
                                         xt[:, c * SQT:(c + 1) * SQT],
                                         start=(kt == 0), stop=(kt == NK - 1))
                        nc.tensor.matmul(v_ps[c], wkv_t[kt // 4][:, kt % 4, HD:2 * HD],
                                         xt[:, c * SQT:(c + 1) * SQT],
                                         start=(kt == 0), stop=(kt == NK - 1))
                for c in range(NSQ):
                    cse = (slice(None), slice(c * SQT, (c + 1) * SQT))
                    rope(k_ps[c], cos_sb[cse], sin_sb[cse], kT_sb[cse], pskv, "pkv")
                    vt_sb = tp.tile([128, SQT], BF16, tag="vt", bufs=2)
                    nc.vector.tensor_copy(out=vt_sb, in_=v_ps[c])
                    for sb in range(SQT // 128):
                        vp = pskv.tile([128, 128], BF16, tag="pkv", name="vtp")
                        nc.tensor.transpose(vp, vt_sb[:, sb * 128:(sb + 1) * 128],
                                            idn_sb)
                        nc.vector.tensor_copy(
                            out=v_sb[:, (4 * c + sb) * 128:(4 * c + sb + 1) * 128],
                            in_=vp)

            # ============ per sq-tile: q-proj -> attention -> AG -> wo =========
            with tc.tile_pool(name="psq", bufs=4, space="PSUM") as psq, \
                 tc.tile_pool(name="pss", bufs=2, space="PSUM") as pss, \
                 tc.tile_pool(name="pso", bufs=1, space="PSUM") as pso, \
                 tc.tile_pool(name="psm", bufs=1, space="PSUM") as psm:

                def qproj(q):
                    s0 = q * SQT
                    cse = (slice(None), slice(s0, s0 + SQT))
                    q_ps = [psq.tile([128, SQT], F32, tag="psq", name=f"qps{h}")
                            for h in range(HL)]
                    for kt in range(NK):
                        xt = xp.tile([128, SQT], BF16, tag="xt")
                        nc.sync.dma_start(
                            out=xt, in_=xT[kt * 128:(kt + 1) * 128, s0:s0 + SQT])
                        for h in range(HL):
                            nc.tensor.matmul(q_ps[h],
                                             wq_t[kt][:, h * 128:(h + 1) * 128],
                                             xt, start=(kt == 0), stop=(kt == NK - 1))
                    return q_ps

                def attention(q, q_ps):
                    s0 = q * SQT
                    cse = (slice(None), slice(s0, s0 + SQT))
                    qT_sb = qp.tile([128, HL, SQT], BF16, tag="qT")
                    nsk_here = 4 * q + 4
                    for h in range(HL):
                        rope(q_ps[h], cos_sb[cse], sin_sb[cse], qT_sb[:, h, :],
                             pss, "s")
                        o_ps = pso.tile([128, SQT], F32, tag="o")
                        sum_ps = psm.tile([128, SQT], F32, tag="sb")

                        def flush(pending):
                            pe, pc, pk = pending
                            nc.tensor.matmul(
                                o_ps[:, pc:], v_sb[:, pk * 128:(pk + 1) * 128],
                                pe[:, pc:], start=(pk == 0),
                                stop=(pk == nsk_here - 1))
                            nc.tensor.matmul(
                                sum_ps[:, pc:], allones, pe[:, pc:],
                                start=(pk == 0), stop=(pk == nsk_here - 1))

                        pending = None  # (e_sb, col0, kt2) awaiting PV+sums
                        for kt2 in range(nsk_here):
                            m = kt2 - 4 * q
                            col0 = 128 * m if m > 0 else 0
                            s_ps = pss.tile([128, SQT], F32, tag="s", name="s_ps")
                            nc.tensor.matmul(
                                s_ps[:, col0:], kT_sb[:, kt2 * 128:(kt2 + 1) * 128],
                                qT_sb[:, h, col0:], start=True, stop=True)
                            if m >= 0:
                                nc.vector.tensor_tensor(
                                    out=s_ps[:, col0:col0 + 128],
                                    in0=s_ps[:, col0:col0 + 128],
                                    in1=mtd_sb[:, kt2, :], op=ALU.add)
                            e_sb = ep.tile([128, SQT], BF16, tag="e")
                            nc.scalar.activation(e_sb[:, col0:], s_ps[:, col0:],
                                                 AF.Exp)
                            if pending is not None:
                                flush(pending)
                            pending = (e_sb, col0, kt2)
                        flush(pending)
                        rec_sb = opool.tile([128, SQT], F32, tag="rcb")
                        nc.vector.reciprocal_approx_fast(out=rec_sb, in_=sum_ps)
                        on_sb = opool.tile([128, SQT], BF16, tag="on")
                        nc.vector.tensor_tensor(out=on_sb, in0=rec_sb, in1=o_ps,
                                                op=ALU.mult)
                        nc.sync.dma_start(out=ag_in[q][h], in_=on_sb)
                        nc.gpsimd.collective_compute(
                            "AllGather", ALU.bypass,
                            replica_groups=[list(range(N_CORES))],
                            ins=[ag_in[q][h].opt()], outs=[ag_out[q][h].opt()])

                def ag_fetch(q):
                    ag_t = []
                    for h in range(HL):
                        t = agp.tile([128, 8, SQT], BF16, tag="ag", bufs=4,
                                     name=f"agt{h}")
                        nc.gpsimd.dma_start(
                            out=t,
                            in_=ag_out[q][h].rearrange("(t p) c -> p t c", p=128))
                        ag_t.append(t)
                    return ag_t

                def wo_phase(q, ag_t):
                    # out rows [q*512, +512) x this core's 512 wo columns
                    for mt in range(4):
                        o1 = psq.tile([128, SQT], F32, tag="psq", name=f"wops{mt}")
                        for h in range(HL):
                            for s in range(8):
                                kt = 4 * s + h
                                nc.tensor.matmul(
                                    o1, ag_t[h][:, s, mt * 128:(mt + 1) * 128],
                                    wo_sb[:, kt, :], start=(h == 0 and s == 0),
                                    stop=(h == HL - 1 and s == 7))
                        ob = outp.tile([128, SQT], F32, tag="ob")
                        nc.scalar.activation(ob, o1, AF.Copy)
                        nc.sync.dma_start(
                            out=out[q * SQT + mt * 128:q * SQT + (mt + 1) * 128, :],
                            in_=ob)

                qps_cur = qproj(0)
                attention(0, qps_cur)
                for q in range(NSQ):
                    if q + 1 < NSQ:
                        qps_cur = qproj(q + 1)
                        ag_t = ag_fetch(q)
                        attention(q + 1, qps_cur)
                    else:
                        ag_t = ag_fetch(q)
                    wo_phase(q, ag_t)

    nc.finalize()
    return nc


_CACHE = {}


def _host_prep(x, wq, wk, wv, wo, cos, sin, mask):
    perm = np.concatenate([np.arange(0, HD, 2), np.arange(1, HD, 2)])
    bf = ml_dtypes.bfloat16
    xT = np.ascontiguousarray(x.reshape(S, D).T).astype(bf)
    cos2 = np.ascontiguousarray(np.vstack([cos.T, cos.T]))
    sin2 = np.ascontiguousarray(np.vstack([sin.T, sin.T]))
    mtd = np.stack([
        np.ascontiguousarray(mask[k * 128:(k + 1) * 128, k * 128:(k + 1) * 128].T)
        for k in range(NSK)])
    swp = np.zeros((128, 128), np.float32)
    for mcol in range(64):
        swp[mcol + 64, mcol] = -1.0
    for mcol in range(64, 128):
        swp[mcol - 64, mcol] = 1.0
    swp = swp.astype(bf)
    idn = np.eye(128, dtype=np.float32).astype(bf)

    scale = 1.0 / math.sqrt(HD)
    in_maps = []
    for c in range(N_CORES):
        qcols = np.concatenate([(4 * c + hh) * HD + perm for hh in range(HL)])
        wq_c = (np.ascontiguousarray(wq[:, qcols]) * np.float32(scale)).astype(bf)
        wkv_c = np.ascontiguousarray(
            np.concatenate([wk[:, c * HD + perm], wv[:, c * HD:(c + 1) * HD]],
                           axis=1)).astype(bf)
        wo_c = np.ascontiguousarray(wo[:, c * SQT:(c + 1) * SQT]).astype(bf)
        in_maps.append({
            "xT": xT, "wq": wq_c, "wkv": wkv_c, "wo": wo_c,
            "cos2": cos2, "sin2": sin2, "mtd": mtd, "swp": swp, "idn": idn,
        })
    return in_maps


def kernel(x, wq, wk, wv, wo, cos, sin, mask, _trace=False):
    in_maps = _host_prep(np.asarray(x, np.float32), np.asarray(wq, np.float32),
                         np.asarray(wk, np.float32), np.asarray(wv, np.float32),
                         np.asarray(wo, np.float32), np.asarray(cos, np.float32),
                         np.asarray(sin, np.float32), np.asarray(mask, np.float32))
    if "nc" not in _CACHE:
        _CACHE["nc"] = build_nc()
    nc = _CACHE["nc"]
    res = run_bass_kernel_spmd(nc, in_maps, core_ids=list(range(N_CORES)),
                               trace=_trace,
                               trace_cores=list(range(N_CORES)) if _trace else None)
    out = np.empty((1, S, D), np.float32)
    for c in range(N_CORES):
        out[0, :, c * SQT:(c + 1) * SQT] = res.results[c]["out"]
    if _trace:
        _CACHE["last_exec_time_ns"] = res.exec_time_ns
        _CACHE["last_results"] = res
    return out
